# revision 2
# baseline (speedup 1.0000x reference)
"""Self-contained Trainium2 Bass kernel for nn_EntAttentionLayer.

Sharding: 8 cores = (batch 4) x (sequence half 2), no collectives.
Each core computes its [1024 tokens, 1024 hid] slice of the output
end-to-end: self-attention (banded mask) -> cross-attention to tag
embeddings -> FFN, each with residual + LayerNorm.

Device layout: activations kept transposed [hid(part), tok(free)].
  - all DMA'd inputs host-staged into [128-partition, ...] contiguous
    layouts so every descriptor moves >=1KB chunks
  - scores computed transposed S^T[k, q]; both head-halves of a pair
    share one [128,1024] PSUM tile so exp runs once per (kt) tile
  - band mask folded post-exp: pt *= e^band on DVE (no PE matmuls)
  - softmax without max subtraction (scores are O(1) for this model)
  - sum_k exp folded into PV matmul via a ones-column appended to V
  - LayerNorm over partitions via bf16 ones-vector matmuls (Pool engine
    makes the bf16 copies); row broadcasts via gpsimd partition_broadcast
Per-core inputs are staged with the sequence ROTATED by half*1024 so all
8 cores run the identical program (band tiles are core-local data).
Matmul operands in bf16 (fp32 PSUM accumulate); residual/LN math in fp32.
"""

import sys

for _p in ("/opt/trn_rl_repo",):
    if _p not in sys.path:
        sys.path.insert(0, _p)

import numpy as np
import ml_dtypes

import concourse.bacc as bacc
import concourse.mybir as mybir
import concourse.tile as tile
from concourse.tile import add_dep_helper
from concourse.bass_utils import run_bass_kernel_spmd

BF = ml_dtypes.bfloat16
fp32 = mybir.dt.float32
bf16 = mybir.dt.bfloat16

H = 1024          # hidden
S = 2048          # full sequence
QL = 1024         # per-core query tokens
FFN = 4096
NH, HD = 16, 64
P = 128
HT = H // P       # 8 hid tiles
ST = S // P       # 16 seq tiles
QN = QL // 512    # 2 q blocks of 512
FT = FFN // P     # 32
FC = 8            # FFN m-tiles per chunk (4 chunks)
T = 50            # tag count
EPS = 1e-12

# pvec column offsets (per-partition param pack, [128, PCOLS] fp32)
QB8, KB, SOB, SLG, SLB = 0, 8, 16, 24, 32
CQB8, CKB, COB, CLG, CLB = 40, 48, 56, 64, 72
IB, OB, OLG, OLB = 80, 112, 120, 128
PCOLS = 136

_CACHE = {}

IDENT = mybir.ActivationFunctionType.Identity


def _band_needed(er):
    """(kt, qn) pairs, in local (rotated) coords, where the band tile can be
    nonzero for either half. Core-independent."""
    out = []
    for kt in range(ST):
        for qn in range(QN):
            lo, hi = qn * 512 - er, qn * 512 + 511 + er
            k0, k1 = kt * P, kt * P + 127
            if (k0 <= hi and k1 >= lo) or (k0 - S <= hi and k1 - S >= lo):
                out.append((kt, qn))
    return out


def _build(er):
    """Build + bacc-compile the per-core program. er = ent_range (>=0)."""
    band_kq = _band_needed(er) if er > 0 else []
    nc = bacc.Bacc()

    # ---- DRAM I/O (all host-pre-transposed for contiguous DMA) ----
    xT = nc.dram_tensor("xT", [P, HT, S], bf16, kind="ExternalInput")
    xh = nc.dram_tensor("xh", [P, HT, QL], fp32, kind="ExternalInput")
    wd = {}
    for n in ("sq", "sk", "sv", "so", "cq", "ck", "cv", "co"):
        wd[n] = nc.dram_tensor(f"w_{n}", [P, HT, HT, P], bf16, kind="ExternalInput")
    w_i = nc.dram_tensor("w_i", [P, FT, HT, P], bf16, kind="ExternalInput")
    w_o = nc.dram_tensor("w_o", [P, HT, FT, P], bf16, kind="ExternalInput")
    pvec = nc.dram_tensor("pvec", [P, PCOLS], fp32, kind="ExternalInput")
    tagsT = nc.dram_tensor("tagsT", [P, HT, 64], bf16, kind="ExternalInput")
    ones_d = nc.dram_tensor("ones", [P, 1], bf16, kind="ExternalInput")
    nb = max(len(band_kq), 1)
    band_d = nc.dram_tensor("band", [P, nb, 512], bf16, kind="ExternalInput")
    yT = nc.dram_tensor("yT", [H, QL], fp32, kind="ExternalOutput")

    with tile.TileContext(nc) as tc:
        with tc.tile_pool(name="p1", bufs=1) as p1, \
             tc.tile_pool(name="p2", bufs=2) as p2, \
             tc.tile_pool(name="p3", bufs=3) as p3, \
             tc.tile_pool(name="p4", bufs=4) as p4, \
             tc.tile_pool(name="psA", bufs=2, space="PSUM") as psA, \
             tc.tile_pool(name="psC", bufs=4, space="PSUM") as psC, \
             tc.tile_pool(name="dram", bufs=1, space="DRAM") as dpool:

            # ---- constants ----
            onest = p1.tile([P, 1], bf16, tag="ones")
            nc.sync.dma_start(out=onest[:], in_=ones_d[:, :])
            pv = p1.tile([P, PCOLS], fp32, tag="pvec")
            nc.sync.dma_start(out=pv[:], in_=pvec[:, :])
            tg = p1.tile([P, HT, 64], bf16, tag="tags")
            nc.sync.dma_start(out=tg[:], in_=tagsT[:, :, :])
            eps_t = p1.tile([1, 1], fp32, tag="eps")
            nc.vector.memset(eps_t[:], EPS)

            def col(c):
                return pv[:, c:c + 1]

            # work: fp32 [128, 8, 1024]; x^T(half) -> t1 -> a -> t2 -> c -> t3 -> y
            work = p1.tile([P, HT, QL], fp32, tag="work")

            # DRAM scratch for K^T and V_aug
            kT_d = dpool.tile([H, S], bf16, name="kT_d")
            vaug = dpool.tile([S, NH, 66], bf16, name="vaug_d")

            def load_w(handle):
                wt = p2.tile([P, HT, HT, P], bf16, tag="w", bufs=2, name="wt")
                for mb in range(HT):
                    nc.sync.dma_start(out=wt[:, mb, :, :], in_=handle[:, mb, :, :])
                return wt

            # ---------- Phase 1: Q^T = ((x_half @ wq) + bq)/8 ----------
            xts = [p2.tile([P, HT, 512], bf16, tag="xs", bufs=2, name=f"xq{i}")
                   for i in range(QN)]
            nc.sync.dma_start(out=xts[0][:], in_=xT[:, :, 0:512])
            w = p2.tile([P, HT, HT, P], bf16, tag="w", bufs=2, name="wt")
            nc.sync.dma_start(out=w[:, 0, :, :], in_=wd["sq"][:, 0, :, :])
            nc.sync.dma_start(out=xts[1][:], in_=xT[:, :, 512:1024])
            for mb in range(1, HT):
                nc.sync.dma_start(out=w[:, mb, :, :], in_=wd["sq"][:, mb, :, :])
            q_sb = p1.tile([P, HT, QL], bf16, tag="qT", name="q_sb")
            for qn in range(QN):
                for mb in range(HT):
                    ps = psA.tile([P, 512], fp32, tag="mm", name="psq")
                    for kt in range(HT):
                        nc.tensor.matmul(ps[:], w[:, mb, kt, :], xts[qn][:, kt, :],
                                         start=(kt == 0), stop=(kt == HT - 1))
                    nc.scalar.activation(out=q_sb[:, mb, qn * 512:(qn + 1) * 512],
                                         in_=ps[:], func=IDENT,
                                         bias=col(QB8 + mb), scale=0.125)

            # ---------- Phase 2: K^T -> DRAM + V -> DRAM (shared x tiles) ----
            wk = load_w(wd["sk"])
            wv = load_w(wd["sv"])
            k_anchor = None
            for sn in range(S // 512):
                xt_ = p2.tile([P, HT, 512], bf16, tag="xs", bufs=2, name="xkv")
                nc.sync.dma_start(out=xt_[:], in_=xT[:, :, sn * 512:(sn + 1) * 512])
                for mb in range(HT):
                    ps = psA.tile([P, 512], fp32, tag="mm", name="psk")
                    for kt in range(HT):
                        mm = nc.tensor.matmul(ps[:], wk[:, mb, kt, :], xt_[:, kt, :],
                                              start=(kt == 0), stop=(kt == HT - 1))
                        if k_anchor is None:
                            k_anchor = mm
                    kt_t = p2.tile([P, 512], bf16, tag="ktmp", name="kt_t")
                    nc.scalar.activation(out=kt_t[:], in_=ps[:], func=IDENT,
                                         bias=col(KB + mb), scale=1.0)
                    nc.sync.dma_start(out=kT_d[mb * P:(mb + 1) * P,
                                               sn * 512:(sn + 1) * 512],
                                      in_=kt_t[:])
                for j in range(4):
                    tt = sn * 4 + j
                    vt = p2.tile([P, NH, 66], bf16, tag="vv", bufs=2, name="vt")
                    for ds in range(2):
                        ps = psA.tile([P, 512], fp32, tag="mm", name="psv")
                        for kt in range(HT):
                            nc.tensor.matmul(ps[:], xt_[:, kt, j * P:(j + 1) * P],
                                             wv[:, 4 * ds:4 * ds + 4, kt, :],
                                             start=(kt == 0), stop=(kt == HT - 1))
                        nc.vector.tensor_copy(
                            out=vt[:, ds * 8:(ds + 1) * 8, 0:64],
                            in_=ps[:].rearrange("p (h c) -> p h c", c=64))
                    nc.gpsimd.memset(vt[:, :, 64:66], 1.0)
                    nc.sync.dma_start(out=vaug[tt * P:(tt + 1) * P, :, :], in_=vt[:])

            # band tiles (e^mask) load on the gpsimd queue, after startup
            band_sb = None
            if band_kq:
                band_sb = p1.tile([P, len(band_kq), 512], bf16, tag="band",
                                  name="band_sb")
                band_dma = nc.gpsimd.dma_start(out=band_sb[:], in_=band_d[:, :, :])
                add_dep_helper(band_dma.ins, k_anchor.ins, sync=True,
                               reason="delay band load past startup")
            band_idx = {kq: i for i, kq in enumerate(band_kq)}

            # ---------- Phase 3: self-attention ----------
            # x residual loads during attention (anchored below), in halves
            work_dmas = [nc.gpsimd.dma_start(out=work[:, 0:4, :], in_=xh[:, 0:4, :]),
                         nc.gpsimd.dma_start(out=work[:, 4:8, :], in_=xh[:, 4:8, :])]
            ctx_sb = p1.tile([P, HT, QL], bf16, tag="ctx", name="ctx_sb")
            att_anchors = {}
            for a in range(NH // 2):
                kp = p2.tile([P, S], bf16, tag="kpair", name="kp")
                for c in range(2):
                    nc.sync.dma_start(out=kp[:, c * 1024:(c + 1) * 1024],
                                      in_=kT_d[a * P:(a + 1) * P,
                                               c * 1024:(c + 1) * 1024])
                vp = p2.tile([P, ST, 2, 66], bf16, tag="vp", name="vp")
                nc.sync.dma_start(out=vp[:], in_=vaug[:, 2 * a:2 * a + 2, :]
                                  .rearrange("(kt p) h c -> p kt h c", p=P))
                for qn in range(QN):
                    cps = [psC.tile([65, 512], fp32, tag="ctx", name=f"ctxps{i}")
                           for i in range(2)]
                    for kt in range(ST):
                        sp = psA.tile([P, 1024], fp32, tag="mm", name="sps")
                        for hh in range(2):
                            mm = nc.tensor.matmul(
                                sp[:, hh * 512:(hh + 1) * 512],
                                kp[hh * 64:(hh + 1) * 64, kt * P:(kt + 1) * P],
                                q_sb[hh * 64:(hh + 1) * 64, a,
                                     qn * 512:(qn + 1) * 512],
                                start=True, stop=True)
                            if a in (1, 2) and (qn, kt, hh) == (0, 0, 0):
                                att_anchors[a] = mm
                        pt = p4.tile([P, 1024], bf16, tag="ptile", bufs=4, name="pt")
                        nc.scalar.activation(out=pt[:], in_=sp[:],
                                             func=mybir.ActivationFunctionType.Exp)
                        if (kt, qn) in band_idx:
                            bi = band_idx[(kt, qn)]
                            for hh in range(2):
                                nc.vector.tensor_mul(
                                    out=pt[:, hh * 512:(hh + 1) * 512],
                                    in0=pt[:, hh * 512:(hh + 1) * 512],
                                    in1=band_sb[:, bi, :])
                        for hh in range(2):
                            nc.tensor.matmul(cps[hh][:], vp[:, kt, hh, 0:65],
                                             pt[:, hh * 512:(hh + 1) * 512],
                                             start=(kt == 0), stop=(kt == ST - 1))
                    for hh in range(2):
                        cp = cps[hh]
                        rec = p3.tile([1, 512], fp32, tag="rows", bufs=2, name="rec")
                        nc.vector.reciprocal(out=rec[:], in_=cp[64:65, :])
                        bc = p2.tile([64, 512], fp32, tag="bc64", name="bca")
                        nc.gpsimd.partition_broadcast(bc[:], rec[0:1, :])
                        nc.vector.tensor_mul(
                            out=ctx_sb[hh * 64:(hh + 1) * 64, a,
                                       qn * 512:(qn + 1) * 512],
                            in0=cp[0:64, :], in1=bc[:])

            for i, (a, mm) in enumerate(sorted(att_anchors.items())):
                add_dep_helper(work_dmas[i].ins, mm.ins, sync=True,
                               reason="residual load rides mid-attention")

            # ---------- residual-add + LayerNorm helpers (transposed) ----------
            def layer_norm(gcol, bcol, out_bf=None):
                """work holds t (fp32). Normalize in place; optional bf16 copy."""
                for qn in range(QN):
                    qs = slice(qn * 512, (qn + 1) * 512)
                    mean_ps = psC.tile([1, 512], fp32, tag="ctx", name="mean_ps")
                    sq_ps = psC.tile([1, 512], fp32, tag="ctx", name="sq_ps")
                    for kt in range(HT):
                        wcp = p3.tile([P, 512], bf16, tag="wcp", bufs=2, name="wcp")
                        nc.gpsimd.tensor_copy(out=wcp[:], in_=work[:, kt, qs])
                        nc.tensor.matmul(mean_ps[:], onest[:], wcp[:],
                                         start=(kt == 0), stop=(kt == HT - 1))
                        sb_ = p2.tile([P, 512], bf16, tag="sqb", name="sb_")
                        nc.scalar.activation(out=sb_[:], in_=work[:, kt, qs],
                                             func=mybir.ActivationFunctionType.Square)
                        nc.tensor.matmul(sq_ps[:], onest[:], sb_[:],
                                         start=(kt == 0), stop=(kt == HT - 1))
                    negmean = p3.tile([1, 512], fp32, tag="rows", bufs=2,
                                      name="negmean")
                    nc.scalar.mul(out=negmean[:], in_=mean_ps[:], mul=-1.0 / H)
                    msq = p3.tile([1, 512], fp32, tag="rows", bufs=2, name="msq")
                    nc.scalar.mul(out=msq[:], in_=sq_ps[:], mul=1.0 / H)
                    nm_bc = p2.tile([P, 512], fp32, tag="bc", name="nm_bc")
                    nc.gpsimd.partition_broadcast(nm_bc[:], negmean[0:1, :])
                    # negmean broadcast issued; square it in place, then var/inv
                    nc.vector.tensor_mul(out=negmean[:], in0=negmean[:],
                                         in1=negmean[:])
                    nc.vector.tensor_sub(out=msq[:], in0=msq[:], in1=negmean[:])
                    nc.scalar.activation(out=msq[:], in_=msq[:],
                                         func=mybir.ActivationFunctionType.Sqrt,
                                         bias=eps_t[:])
                    nc.vector.reciprocal(out=msq[:], in_=msq[:])
                    iv_bc = p2.tile([P, 512], fp32, tag="bc", name="iv_bc")
                    nc.gpsimd.partition_broadcast(iv_bc[:], msq[0:1, :])
                    for j in range(HT):
                        nc.vector.tensor_add(out=work[:, j, qs], in0=work[:, j, qs],
                                             in1=nm_bc[:])
                        nc.vector.tensor_mul(out=work[:, j, qs], in0=work[:, j, qs],
                                             in1=iv_bc[:])
                        nc.scalar.activation(out=work[:, j, qs], in_=work[:, j, qs],
                                             func=IDENT,
                                             bias=col(bcol + j), scale=col(gcol + j))
                        if out_bf is not None:
                            nc.vector.tensor_copy(out=out_bf[:, j, qs],
                                                  in_=work[:, j, qs])

            def proj_add_residual(wt, rhs, bcol):
                """work <- (proj of rhs via wt) + bias + work, per [mb, qn] tile."""
                for mb in range(HT):
                    for qn in range(QN):
                        qs = slice(qn * 512, (qn + 1) * 512)
                        ps = psA.tile([P, 512], fp32, tag="mm", name="pso")
                        for kt in range(HT):
                            nc.tensor.matmul(ps[:], wt[:, mb, kt, :], rhs[:, kt, qs],
                                             start=(kt == 0), stop=(kt == HT - 1))
                        nc.vector.scalar_tensor_tensor(
                            out=work[:, mb, qs], in0=ps[:], scalar=col(bcol + mb),
                            in1=work[:, mb, qs],
                            op0=mybir.AluOpType.add, op1=mybir.AluOpType.add)

            # ---------- Phase 4: self out-proj + residual + LN1 ----------
            wso = load_w(wd["so"])
            # cross-attention K (tiny; weight prefetches under attention)
            wck = load_w(wd["ck"])
            kc = p1.tile([P, HT, 64], bf16, tag="kc", name="kc")
            for mb in range(HT):
                ps = psA.tile([P, T], fp32, tag="mm", name="pskc")
                for kt in range(HT):
                    nc.tensor.matmul(ps[:], wck[:, mb, kt, :], tg[:, kt, 0:T],
                                     start=(kt == 0), stop=(kt == HT - 1))
                nc.scalar.activation(out=kc[:, mb, 0:T], in_=ps[:], func=IDENT,
                                     bias=col(CKB + mb), scale=1.0)

            proj_add_residual(wso, ctx_sb, SOB)
            a_bf = p1.tile([P, HT, QL], bf16, tag="qT", name="a_bf")
            layer_norm(SLG, SLB, out_bf=a_bf)

            # ---------- Phase 5: cross-attention ----------
            wcv = load_w(wd["cv"])
            vca = p1.tile([P, NH, 66], bf16, tag="vca", name="vca")
            for ds in range(2):
                ps = psA.tile([T, 512], fp32, tag="mm", name="psvc")
                for kt in range(HT):
                    nc.tensor.matmul(ps[:], tg[:, kt, 0:T],
                                     wcv[:, 4 * ds:4 * ds + 4, kt, :],
                                     start=(kt == 0), stop=(kt == HT - 1))
                nc.vector.tensor_copy(out=vca[0:T, ds * 8:(ds + 1) * 8, 0:64],
                                      in_=ps[:].rearrange("p (h c) -> p h c", c=64))
            nc.gpsimd.memset(vca[0:T, :, 64:66], 1.0)

            # fused: per (qn, a): q-proj -> scores -> exp -> PV -> normalize
            wcq = load_w(wd["cq"])
            ctxc = p1.tile([P, HT, QL], bf16, tag="ctx", name="ctxc")
            for qn in range(QN):
                qs = slice(qn * 512, (qn + 1) * 512)
                for a in range(HT):
                    ps = psA.tile([P, 512], fp32, tag="mm", name="psqc")
                    for kt in range(HT):
                        nc.tensor.matmul(ps[:], wcq[:, a, kt, :], a_bf[:, kt, qs],
                                         start=(kt == 0), stop=(kt == HT - 1))
                    qc_t = p2.tile([P, 512], bf16, tag="ktmp", name="qc_t")
                    nc.scalar.activation(out=qc_t[:], in_=ps[:], func=IDENT,
                                         bias=col(CQB8 + a), scale=0.125)
                    spc = psA.tile([T, 1024], fp32, tag="mm", name="spc")
                    for hh in range(2):
                        nc.tensor.matmul(spc[:, hh * 512:(hh + 1) * 512],
                                         kc[hh * 64:(hh + 1) * 64, a, 0:T],
                                         qc_t[hh * 64:(hh + 1) * 64, :],
                                         start=True, stop=True)
                    ptc = p4.tile([T, 1024], bf16, tag="ptile", bufs=4, name="ptc")
                    nc.scalar.activation(out=ptc[:], in_=spc[:],
                                         func=mybir.ActivationFunctionType.Exp)
                    for hh in range(2):
                        cp = psC.tile([65, 512], fp32, tag="ctx", name="cpc")
                        nc.tensor.matmul(cp[:], vca[0:T, 2 * a + hh, 0:65],
                                         ptc[:, hh * 512:(hh + 1) * 512],
                                         start=True, stop=True)
                        rec = p3.tile([1, 512], fp32, tag="rows", bufs=2,
                                      name="recc")
                        nc.vector.reciprocal(out=rec[:], in_=cp[64:65, :])
                        bcc = p2.tile([64, 512], fp32, tag="bc64", name="bcc")
                        nc.gpsimd.partition_broadcast(bcc[:], rec[0:1, :])
                        nc.vector.tensor_mul(
                            out=ctxc[hh * 64:(hh + 1) * 64, a, qs],
                            in0=cp[0:64, :], in1=bcc[:])

            # ---------- Phase 6: cross out-proj + residual + LN2 ----------
            wco = load_w(wd["co"])
            proj_add_residual(wco, ctxc, COB)
            c_bf = p1.tile([P, HT, QL], bf16, tag="act_bf", name="c_bf")
            layer_norm(CLG, CLB, out_bf=c_bf)

            # ---------- Phase 7: FFN (chunk-outer: each weight block loads once) --
            for ch in range(FT // FC):
                inters = [p2.tile([P, HT, 512], bf16, tag="xs", bufs=2,
                                  name=f"inter{i}") for i in range(QN)]
                for mi in range(FC):
                    m = ch * FC + mi
                    wi = p3.tile([P, HT, P], bf16, tag="wi", bufs=4, name="wi")
                    nc.sync.dma_start(out=wi[:], in_=w_i[:, m, :, :])
                    for qn in range(QN):
                        qs = slice(qn * 512, (qn + 1) * 512)
                        ps = psA.tile([P, 512], fp32, tag="mm", name="psi")
                        for kt in range(HT):
                            nc.tensor.matmul(ps[:], wi[:, kt, :], c_bf[:, kt, qs],
                                             start=(kt == 0), stop=(kt == HT - 1))
                        nc.scalar.activation(out=inters[qn][:, mi, :], in_=ps[:],
                                             func=mybir.ActivationFunctionType.Gelu,
                                             bias=col(IB + m), scale=1.0)
                for mo in range(HT):
                    wo = p2.tile([P, FC, P], bf16, tag="wo", bufs=4, name="wo")
                    nc.sync.dma_start(out=wo[:],
                                      in_=w_o[:, mo, ch * FC:(ch + 1) * FC, :])
                    for qn in range(QN):
                        qs = slice(qn * 512, (qn + 1) * 512)
                        ps = psA.tile([P, 512], fp32, tag="mm", name="pso2")
                        for kt in range(FC):
                            nc.tensor.matmul(ps[:], wo[:, kt, :],
                                             inters[qn][:, kt, :],
                                             start=(kt == 0), stop=(kt == FC - 1))
                        if ch == 0:
                            nc.vector.scalar_tensor_tensor(
                                out=work[:, mo, qs], in0=ps[:], scalar=col(OB + mo),
                                in1=work[:, mo, qs],
                                op0=mybir.AluOpType.add, op1=mybir.AluOpType.add)
                        else:
                            nc.vector.tensor_add(out=work[:, mo, qs], in0=ps[:],
                                                 in1=work[:, mo, qs])

            layer_norm(OLG, OLB)
            for qn in range(QN):
                qs = slice(qn * 512, (qn + 1) * 512)
                for j in range(HT):
                    nc.sync.dma_start(out=yT[j * P:(j + 1) * P, qs],
                                      in_=work[:, j, qs])

    nc.compile()
    return nc, band_kq


def _get_program(er):
    key = int(er)
    if key not in _CACHE:
        _CACHE[key] = _build(key)
    return _CACHE[key]


def _tr_w(w, ktt, mbt):
    """[K, M] -> [P, mb, kt, 128] so per-mb DMAs read 2KB/partition chunks."""
    return np.ascontiguousarray(
        np.asarray(w, np.float32).reshape(ktt, P, mbt, P).transpose(1, 2, 0, 3)
    ).astype(BF)


def build_in_maps(inp, band_kq, er):
    x = inp["x"].astype(np.float32)
    B, S_, H_ = x.shape

    # host-side shared staging
    wcast = {}
    for n in ("sq", "sk", "sv", "so", "cq", "ck", "cv", "co"):
        wcast[n] = _tr_w(inp[n + "_w"], HT, HT)
    wcast["i"] = _tr_w(inp["i_w"], HT, FT)
    wcast["o"] = _tr_w(inp["o_w"], FT, HT)

    so_b_eff = inp["so_b"].astype(np.float32) + \
        inp["sv_b"].astype(np.float32) @ inp["so_w"].astype(np.float32)
    co_b_eff = inp["co_b"].astype(np.float32) + \
        inp["cv_b"].astype(np.float32) @ inp["co_w"].astype(np.float32)
    pvec = np.zeros((P, PCOLS), np.float32)

    def pack(colbase, vec):
        v = np.asarray(vec, np.float32).reshape(-1, P)  # [k, 128]
        pvec[:, colbase:colbase + v.shape[0]] = v.T

    pack(QB8, inp["sq_b"].astype(np.float32) * 0.125)
    pack(KB, inp["sk_b"])
    pack(SOB, so_b_eff)
    pack(SLG, inp["sln_g"]); pack(SLB, inp["sln_b"])
    pack(CQB8, inp["cq_b"].astype(np.float32) * 0.125)
    pack(CKB, inp["ck_b"])
    pack(COB, co_b_eff)
    pack(CLG, inp["cln_g"]); pack(CLB, inp["cln_b"])
    pack(IB, inp["i_b"])
    pack(OB, inp["o_b"])
    pack(OLG, inp["oln_g"]); pack(OLB, inp["oln_b"])

    tags = inp["emb_table"].astype(np.float32)[
        np.asarray(inp["ent_ids"]).astype(np.int64)]  # [T, H]
    assert tags.shape[0] == T, f"program compiled for {T} tags, got {tags.shape[0]}"
    tagsT = np.zeros((H, 64), np.float32)
    tagsT[:, :tags.shape[0]] = tags.T
    tagsT3 = np.ascontiguousarray(
        tagsT.reshape(HT, P, 64).transpose(1, 0, 2)).astype(BF)
    ones = np.ones((P, 1), BF)

    # band tiles (e^mask) in local (rotated) coords, per half
    nb = max(len(band_kq), 1)
    band_h = np.ones((2, P, nb, 512), np.float32)
    if band_kq and er > 0:
        for i, (kt, qn) in enumerate(band_kq):
            k_rot = kt * P + np.arange(P)[:, None]
            q_rot = qn * 512 + np.arange(512)[None, :]
            d = k_rot - q_rot
            m0 = (np.abs(d) <= er).astype(np.float32)
            m1 = np.where(k_rot >= S_ - QL,
                          (np.abs(d - S_) <= er).astype(np.float32), m0)
            band_h[0, :, i, :] = np.exp(m0)
            band_h[1, :, i, :] = np.exp(m1)
    band_h = band_h.astype(BF)

    in_maps = []
    for c in range(8):
        b, half = divmod(c, 2)
        xt = x[b].T  # [H, S]
        rot = np.concatenate([xt[:, half * QL:], xt[:, :half * QL]], axis=1)
        xt3 = np.ascontiguousarray(
            rot.reshape(HT, P, S_).transpose(1, 0, 2)).astype(BF)
        xh3 = np.ascontiguousarray(
            rot[:, :QL].reshape(HT, P, QL).transpose(1, 0, 2))
        in_maps.append({
            "xT": xt3,
            "xh": xh3,
            "w_sq": wcast["sq"], "w_sk": wcast["sk"], "w_sv": wcast["sv"],
            "w_so": wcast["so"], "w_cq": wcast["cq"], "w_ck": wcast["ck"],
            "w_cv": wcast["cv"], "w_co": wcast["co"],
            "w_i": wcast["i"], "w_o": wcast["o"],
            "pvec": pvec, "tagsT": tagsT3, "ones": ones,
            "band": np.ascontiguousarray(band_h[half]),
        })
    return in_maps


def kernel(**inputs):
    inp = {k: np.asarray(v) for k, v in inputs.items()}
    x = inp["x"]
    B, S_, H_ = x.shape
    er = int(inp["ent_range"])
    nc, band_kq = _get_program(er)
    in_maps = build_in_maps(inp, band_kq, er)

    res = run_bass_kernel_spmd(nc, in_maps, core_ids=list(range(8)))
    out = np.empty((B, S_, H_), np.float32)
    for c in range(8):
        b, half = divmod(c, 2)
        out[b, half * QL:(half + 1) * QL, :] = res.results[c]["yT"].T
    return out


# revision 10
# speedup vs baseline: 1.1070x; 1.1070x over previous
"""Self-contained Trainium2 Bass kernel for nn_EntAttentionLayer.

Sharding: 8 cores = (batch 4) x (sequence half 2), no collectives.
Each core computes its [1024 tokens, 1024 hid] slice of the output
end-to-end: self-attention (banded mask) -> cross-attention to tag
embeddings -> FFN, each with residual + LayerNorm.

Device layout: activations kept transposed [hid(part), tok(free)].
  - all DMA'd inputs host-staged into [128-partition, ...] contiguous
    layouts so every descriptor moves >=1KB chunks
  - scores computed transposed S^T[k, q]; both head-halves of a pair
    share one [128,1024] PSUM tile so exp runs once per (kt) tile
  - band mask folded post-exp: pt *= e^band on DVE (no PE matmuls)
  - softmax without max subtraction (scores are O(1) for this model)
  - sum_k exp folded into PV matmul via a ones-column appended to V
  - LayerNorm over partitions via bf16 ones-vector matmuls (Pool engine
    makes the bf16 copies); row broadcasts via gpsimd partition_broadcast
Per-core inputs are staged with the sequence ROTATED by half*1024 so all
8 cores run the identical program (band tiles are core-local data).
Matmul operands in bf16 (fp32 PSUM accumulate); residual/LN math in fp32.
"""

import sys

for _p in ("/opt/trn_rl_repo",):
    if _p not in sys.path:
        sys.path.insert(0, _p)

import numpy as np
import ml_dtypes

import concourse.bacc as bacc
import concourse.mybir as mybir
import concourse.tile as tile
from concourse.tile import add_dep_helper
from concourse.bass_utils import run_bass_kernel_spmd

BF = ml_dtypes.bfloat16
fp32 = mybir.dt.float32
bf16 = mybir.dt.bfloat16

H = 1024          # hidden
S = 2048          # full sequence
QL = 1024         # per-core query tokens
FFN = 4096
NH, HD = 16, 64
P = 128
HT = H // P       # 8 hid tiles
ST = S // P       # 16 seq tiles
QN = QL // 512    # 2 q blocks of 512
FT = FFN // P     # 32
FC = 8            # FFN m-tiles per chunk (4 chunks)
T = 50            # tag count
EPS = 1e-12

# pvec column offsets (per-partition param pack, [128, PCOLS] fp32)
QB8, KB, SOB, SLG, SLB = 0, 8, 16, 24, 32
CQB8, CKB, COB, CLG, CLB = 40, 48, 56, 64, 72
IB, OB, OLG, OLB = 80, 112, 120, 128
ONECOL = 136  # column of fp32 ones (LN mean matmul lhsT)
PCOLS = 137

_CACHE = {}

IDENT = mybir.ActivationFunctionType.Identity


def _band_needed(er):
    """(kt, qn) pairs, in local (rotated) coords, where the band tile can be
    nonzero for either half. Core-independent."""
    out = []
    for kt in range(ST):
        for qn in range(QN):
            lo, hi = qn * 512 - er, qn * 512 + 511 + er
            k0, k1 = kt * P, kt * P + 127
            if (k0 <= hi and k1 >= lo) or (k0 - S <= hi and k1 - S >= lo):
                out.append((kt, qn))
    return out


def _build(er):
    """Build + bacc-compile the per-core program. er = ent_range (>=0)."""
    band_kq = _band_needed(er) if er > 0 else []
    nc = bacc.Bacc()

    # ---- DRAM I/O (all host-pre-transposed for contiguous DMA) ----
    xT = nc.dram_tensor("xT", [P, HT, S], bf16, kind="ExternalInput")
    xh = nc.dram_tensor("xh", [P, HT, QL], fp32, kind="ExternalInput")
    wd = {}
    for n in ("sq", "sk", "sv", "so", "cq", "ck", "cv", "co"):
        wd[n] = nc.dram_tensor(f"w_{n}", [P, HT, HT, P], bf16, kind="ExternalInput")
    w_i = nc.dram_tensor("w_i", [P, FT, HT, P], bf16, kind="ExternalInput")
    w_o = nc.dram_tensor("w_o", [P, HT, FT, P], bf16, kind="ExternalInput")
    pvec = nc.dram_tensor("pvec", [P, PCOLS], fp32, kind="ExternalInput")
    tagsT = nc.dram_tensor("tagsT", [P, HT, 64], bf16, kind="ExternalInput")
    ones_d = nc.dram_tensor("ones", [P, 1], bf16, kind="ExternalInput")
    nb = max(len(band_kq), 1)
    band_d = nc.dram_tensor("band", [P, nb, 512], bf16, kind="ExternalInput")
    yT = nc.dram_tensor("yT", [H, QL], fp32, kind="ExternalOutput")

    with tile.TileContext(nc) as tc:
        with tc.tile_pool(name="p1", bufs=1) as p1, \
             tc.tile_pool(name="p2", bufs=2) as p2, \
             tc.tile_pool(name="p3", bufs=3) as p3, \
             tc.tile_pool(name="p4", bufs=4) as p4, \
             tc.tile_pool(name="psA", bufs=2, space="PSUM") as psA, \
             tc.tile_pool(name="psC", bufs=4, space="PSUM") as psC, \
             tc.tile_pool(name="dram", bufs=1, space="DRAM") as dpool, \
             tc.tile_pool(name="dscr", bufs=4, space="DRAM") as dscr:

            # ---- constants ----
            onest = p1.tile([P, 1], bf16, tag="ones")
            nc.sync.dma_start(out=onest[:], in_=ones_d[:, :])
            pv = p1.tile([P, PCOLS], fp32, tag="pvec")
            nc.sync.dma_start(out=pv[:], in_=pvec[:, :])
            tg = p1.tile([P, HT, 64], bf16, tag="tags")
            nc.sync.dma_start(out=tg[:], in_=tagsT[:, :, :])
            eps_t = p1.tile([1, 1], fp32, tag="eps")
            nc.vector.memset(eps_t[:], EPS)

            def col(c):
                return pv[:, c:c + 1]

            # work: fp32 [128, 8, 1024]; x^T(half) -> t1 -> a -> t2 -> c -> t3 -> y
            work = p1.tile([P, HT, QL], fp32, tag="work")

            # DRAM scratch for K^T and V_aug
            kT_d = dpool.tile([H, S], bf16, name="kT_d")
            vaug = dpool.tile([S, NH, 66], bf16, name="vaug_d")

            def load_w(handle):
                wt = p2.tile([P, HT, HT, P], bf16, tag="w", bufs=2, name="wt")
                for mb in range(HT):
                    nc.sync.dma_start(out=wt[:, mb, :, :], in_=handle[:, mb, :, :])
                return wt

            # ---------- Phase 1: Q^T = ((x_half @ wq) + bq)/8 ----------
            xts = [p2.tile([P, HT, 512], bf16, tag="xs", bufs=2, name=f"xq{i}")
                   for i in range(QN)]
            nc.sync.dma_start(out=xts[0][:], in_=xT[:, :, 0:512])
            w = p2.tile([P, HT, HT, P], bf16, tag="w", bufs=2, name="wt")
            nc.sync.dma_start(out=w[:, 0, :, :], in_=wd["sq"][:, 0, :, :])
            nc.sync.dma_start(out=xts[1][:], in_=xT[:, :, 512:1024])
            for mb in range(1, HT):
                nc.sync.dma_start(out=w[:, mb, :, :], in_=wd["sq"][:, mb, :, :])
            q_sb = p1.tile([P, HT, QL], bf16, tag="qT", name="q_sb")
            for qn in range(QN):
                for mb in range(HT):
                    ps = psA.tile([P, 512], fp32, tag="mm", name="psq")
                    for kt in range(HT):
                        nc.tensor.matmul(ps[:], w[:, mb, kt, :], xts[qn][:, kt, :],
                                         start=(kt == 0), stop=(kt == HT - 1))
                    nc.scalar.activation(out=q_sb[:, mb, qn * 512:(qn + 1) * 512],
                                         in_=ps[:], func=IDENT,
                                         bias=col(QB8 + mb), scale=0.125)

            # ---------- Phase 2: K^T -> DRAM + V -> DRAM (shared x tiles) ----
            wk = load_w(wd["sk"])
            wv = load_w(wd["sv"])
            k_anchor = None
            for sn in range(S // 512):
                xt_ = p2.tile([P, HT, 512], bf16, tag="xs", bufs=2, name="xkv")
                nc.sync.dma_start(out=xt_[:], in_=xT[:, :, sn * 512:(sn + 1) * 512])
                for mb in range(HT):
                    ps = psA.tile([P, 512], fp32, tag="mm", name="psk")
                    for kt in range(HT):
                        mm = nc.tensor.matmul(ps[:], wk[:, mb, kt, :], xt_[:, kt, :],
                                              start=(kt == 0), stop=(kt == HT - 1))
                        if k_anchor is None:
                            k_anchor = mm
                    kt_t = p2.tile([P, 512], bf16, tag="ktmp", name="kt_t")
                    nc.scalar.activation(out=kt_t[:], in_=ps[:], func=IDENT,
                                         bias=col(KB + mb), scale=1.0)
                    nc.sync.dma_start(out=kT_d[mb * P:(mb + 1) * P,
                                               sn * 512:(sn + 1) * 512],
                                      in_=kt_t[:])
                for j in range(4):
                    tt = sn * 4 + j
                    vt = p2.tile([P, NH, 66], bf16, tag="vv", bufs=2, name="vt")
                    for ds in range(2):
                        ps = psA.tile([P, 512], fp32, tag="mm", name="psv")
                        for kt in range(HT):
                            nc.tensor.matmul(ps[:], xt_[:, kt, j * P:(j + 1) * P],
                                             wv[:, 4 * ds:4 * ds + 4, kt, :],
                                             start=(kt == 0), stop=(kt == HT - 1))
                        nc.vector.tensor_copy(
                            out=vt[:, ds * 8:(ds + 1) * 8, 0:64],
                            in_=ps[:].rearrange("p (h c) -> p h c", c=64))
                    nc.vector.memset(vt[:, :, 64:66], 1.0)
                    nc.sync.dma_start(out=vaug[tt * P:(tt + 1) * P, :, :], in_=vt[:])

            # band tiles (e^mask) load on the gpsimd queue, after startup
            band_sb = None
            if band_kq:
                band_sb = p1.tile([P, len(band_kq), 512], bf16, tag="band",
                                  name="band_sb")
                band_dma = nc.gpsimd.dma_start(out=band_sb[:], in_=band_d[:, :, :])
                add_dep_helper(band_dma.ins, k_anchor.ins, sync=True,
                               reason="delay band load past startup")
            band_idx = {kq: i for i, kq in enumerate(band_kq)}

            # ---------- Phase 3: self-attention ----------
            # x residual loads during attention (anchored below), in halves
            work_dmas = [nc.gpsimd.dma_start(out=work[:, 0:4, :], in_=xh[:, 0:4, :]),
                         nc.gpsimd.dma_start(out=work[:, 4:8, :], in_=xh[:, 4:8, :])]
            ctx_sb = p1.tile([P, HT, QL], bf16, tag="ctx", name="ctx_sb")
            att_anchors = {}
            for a in range(NH // 2):
                kp = p2.tile([P, S], bf16, tag="kpair", name="kp")
                for c in range(2):
                    nc.sync.dma_start(out=kp[:, c * 1024:(c + 1) * 1024],
                                      in_=kT_d[a * P:(a + 1) * P,
                                               c * 1024:(c + 1) * 1024])
                vp = p2.tile([P, ST, 2, 66], bf16, tag="vp", name="vp")
                nc.sync.dma_start(out=vp[:], in_=vaug[:, 2 * a:2 * a + 2, :]
                                  .rearrange("(kt p) h c -> p kt h c", p=P))
                for qn in range(QN):
                    cps = [psC.tile([65, 512], fp32, tag="ctx", name=f"ctxps{i}")
                           for i in range(2)]
                    for kt in range(ST):
                        sp = psA.tile([P, 1024], fp32, tag="mm", name="sps")
                        for hh in range(2):
                            mm = nc.tensor.matmul(
                                sp[:, hh * 512:(hh + 1) * 512],
                                kp[hh * 64:(hh + 1) * 64, kt * P:(kt + 1) * P],
                                q_sb[hh * 64:(hh + 1) * 64, a,
                                     qn * 512:(qn + 1) * 512],
                                start=True, stop=True)
                            if a in (1, 2) and (qn, kt, hh) == (0, 0, 0):
                                att_anchors[a] = mm
                        pt = p4.tile([P, 1024], bf16, tag="ptile", bufs=4, name="pt")
                        nc.scalar.activation(out=pt[:], in_=sp[:],
                                             func=mybir.ActivationFunctionType.Exp)
                        if (kt, qn) in band_idx:
                            bi = band_idx[(kt, qn)]
                            for hh in range(2):
                                nc.vector.tensor_mul(
                                    out=pt[:, hh * 512:(hh + 1) * 512],
                                    in0=pt[:, hh * 512:(hh + 1) * 512],
                                    in1=band_sb[:, bi, :])
                        for hh in range(2):
                            nc.tensor.matmul(cps[hh][:], vp[:, kt, hh, 0:65],
                                             pt[:, hh * 512:(hh + 1) * 512],
                                             start=(kt == 0), stop=(kt == ST - 1))
                    for hh in range(2):
                        cp = cps[hh]
                        rec = p3.tile([1, 512], fp32, tag="rows", bufs=2, name="rec")
                        nc.vector.reciprocal(out=rec[:], in_=cp[64:65, :])
                        scr = dscr.tile([1, 512], fp32, name="scr_a")
                        nc.sync.dma_start(out=scr[:, :], in_=rec[:])
                        bc = p2.tile([64, 512], fp32, tag="bc64", name="bca")
                        nc.sync.dma_start(out=bc[:], in_=scr[0:1, :].partition_broadcast(64))
                        nc.vector.tensor_mul(
                            out=ctx_sb[hh * 64:(hh + 1) * 64, a,
                                       qn * 512:(qn + 1) * 512],
                            in0=cp[0:64, :], in1=bc[:])

            for i, (a, mm) in enumerate(sorted(att_anchors.items())):
                add_dep_helper(work_dmas[i].ins, mm.ins, sync=True,
                               reason="residual load rides mid-attention")

            # ---------- residual-add + LayerNorm helpers (transposed) ----------
            def layer_norm(gcol, bcol, out_bf=None):
                """work holds t (fp32). Normalize in place; optional bf16 copy."""
                for qn in range(QN):
                    qs = slice(qn * 512, (qn + 1) * 512)
                    mean_ps = psC.tile([1, 512], fp32, tag="ctx", name="mean_ps")
                    sq_ps = psC.tile([1, 512], fp32, tag="ctx", name="sq_ps")
                    for kt in range(HT):
                        # mean directly from fp32 work (no copy step on the chain)
                        nc.tensor.matmul(mean_ps[:], col(ONECOL), work[:, kt, qs],
                                         start=(kt == 0), stop=(kt == HT - 1))
                        sb_ = p2.tile([P, 512], bf16, tag="sqb", name="sb_")
                        nc.scalar.activation(out=sb_[:], in_=work[:, kt, qs],
                                             func=mybir.ActivationFunctionType.Square)
                        nc.tensor.matmul(sq_ps[:], onest[:], sb_[:],
                                         start=(kt == 0), stop=(kt == HT - 1))
                    negmean = p3.tile([1, 512], fp32, tag="rows", bufs=2,
                                      name="negmean")
                    nc.scalar.mul(out=negmean[:], in_=mean_ps[:], mul=-1.0 / H)
                    msq = p3.tile([1, 512], fp32, tag="rows", bufs=2, name="msq")
                    nc.scalar.mul(out=msq[:], in_=sq_ps[:], mul=1.0 / H)
                    scr = dscr.tile([2, 512], fp32, name="scr_ln")
                    nc.sync.dma_start(out=scr[0:1, :], in_=negmean[:])
                    # negmean shipped; square it in place, then var/std/inv in msq
                    nc.vector.tensor_mul(out=negmean[:], in0=negmean[:],
                                         in1=negmean[:])
                    nc.vector.tensor_sub(out=msq[:], in0=msq[:], in1=negmean[:])
                    nc.scalar.activation(out=msq[:], in_=msq[:],
                                         func=mybir.ActivationFunctionType.Sqrt,
                                         bias=eps_t[:])
                    nc.vector.reciprocal(out=msq[:], in_=msq[:])
                    nc.sync.dma_start(out=scr[1:2, :], in_=msq[:])
                    nm_bc = p2.tile([P, 512], fp32, tag="bc", name="nm_bc")
                    nc.sync.dma_start(out=nm_bc[:], in_=scr[0:1, :].partition_broadcast(P))
                    iv_bc = p2.tile([P, 512], fp32, tag="bc", name="iv_bc")
                    nc.sync.dma_start(out=iv_bc[:], in_=scr[1:2, :].partition_broadcast(P))
                    for j in range(HT):
                        nc.vector.tensor_add(out=work[:, j, qs], in0=work[:, j, qs],
                                             in1=nm_bc[:])
                        nc.vector.tensor_mul(out=work[:, j, qs], in0=work[:, j, qs],
                                             in1=iv_bc[:])
                        nc.scalar.activation(out=work[:, j, qs], in_=work[:, j, qs],
                                             func=IDENT,
                                             bias=col(bcol + j), scale=col(gcol + j))
                        if out_bf is not None:
                            nc.vector.tensor_copy(out=out_bf[:, j, qs],
                                                  in_=work[:, j, qs])

            def proj_add_residual(wt, rhs, bcol):
                """work <- (proj of rhs via wt) + bias + work, per [mb, qn] tile."""
                for mb in range(HT):
                    for qn in range(QN):
                        qs = slice(qn * 512, (qn + 1) * 512)
                        ps = psA.tile([P, 512], fp32, tag="mm", name="pso")
                        for kt in range(HT):
                            nc.tensor.matmul(ps[:], wt[:, mb, kt, :], rhs[:, kt, qs],
                                             start=(kt == 0), stop=(kt == HT - 1))
                        nc.vector.scalar_tensor_tensor(
                            out=work[:, mb, qs], in0=ps[:], scalar=col(bcol + mb),
                            in1=work[:, mb, qs],
                            op0=mybir.AluOpType.add, op1=mybir.AluOpType.add)

            # ---------- Phase 4: self out-proj + residual + LN1 ----------
            wso = load_w(wd["so"])
            # cross-attention K (tiny; weight prefetches under attention)
            wck = load_w(wd["ck"])
            kc = p1.tile([P, HT, 64], bf16, tag="kc", name="kc")
            for mb in range(HT):
                ps = psA.tile([P, T], fp32, tag="mm", name="pskc")
                for kt in range(HT):
                    nc.tensor.matmul(ps[:], wck[:, mb, kt, :], tg[:, kt, 0:T],
                                     start=(kt == 0), stop=(kt == HT - 1))
                nc.scalar.activation(out=kc[:, mb, 0:T], in_=ps[:], func=IDENT,
                                     bias=col(CKB + mb), scale=1.0)

            proj_add_residual(wso, ctx_sb, SOB)
            a_bf = p1.tile([P, HT, QL], bf16, tag="qT", name="a_bf")
            layer_norm(SLG, SLB, out_bf=a_bf)

            # ---------- Phase 5: cross-attention ----------
            wcv = load_w(wd["cv"])
            vca = p1.tile([P, NH, 66], bf16, tag="vca", name="vca")
            for ds in range(2):
                ps = psA.tile([T, 512], fp32, tag="mm", name="psvc")
                for kt in range(HT):
                    nc.tensor.matmul(ps[:], tg[:, kt, 0:T],
                                     wcv[:, 4 * ds:4 * ds + 4, kt, :],
                                     start=(kt == 0), stop=(kt == HT - 1))
                nc.vector.tensor_copy(out=vca[0:T, ds * 8:(ds + 1) * 8, 0:64],
                                      in_=ps[:].rearrange("p (h c) -> p h c", c=64))
            nc.vector.memset(vca[0:T, :, 64:66], 1.0)

            # fused: per (qn, a): q-proj -> scores -> exp -> PV -> normalize
            wcq = load_w(wd["cq"])
            ctxc = p1.tile([P, HT, QL], bf16, tag="ctx", name="ctxc")
            for qn in range(QN):
                qs = slice(qn * 512, (qn + 1) * 512)
                for a in range(HT):
                    ps = psA.tile([P, 512], fp32, tag="mm", name="psqc")
                    for kt in range(HT):
                        nc.tensor.matmul(ps[:], wcq[:, a, kt, :], a_bf[:, kt, qs],
                                         start=(kt == 0), stop=(kt == HT - 1))
                    qc_t = p2.tile([P, 512], bf16, tag="ktmp", name="qc_t")
                    nc.scalar.activation(out=qc_t[:], in_=ps[:], func=IDENT,
                                         bias=col(CQB8 + a), scale=0.125)
                    spc = psA.tile([T, 1024], fp32, tag="mm", name="spc")
                    for hh in range(2):
                        nc.tensor.matmul(spc[:, hh * 512:(hh + 1) * 512],
                                         kc[hh * 64:(hh + 1) * 64, a, 0:T],
                                         qc_t[hh * 64:(hh + 1) * 64, :],
                                         start=True, stop=True)
                    ptc = p4.tile([T, 1024], bf16, tag="ptile", bufs=4, name="ptc")
                    nc.scalar.activation(out=ptc[:], in_=spc[:],
                                         func=mybir.ActivationFunctionType.Exp)
                    for hh in range(2):
                        cp = psC.tile([65, 512], fp32, tag="ctx", name="cpc")
                        nc.tensor.matmul(cp[:], vca[0:T, 2 * a + hh, 0:65],
                                         ptc[:, hh * 512:(hh + 1) * 512],
                                         start=True, stop=True)
                        rec = p3.tile([1, 512], fp32, tag="rows", bufs=2,
                                      name="recc")
                        nc.vector.reciprocal(out=rec[:], in_=cp[64:65, :])
                        scr = dscr.tile([1, 512], fp32, name="scr_c")
                        nc.sync.dma_start(out=scr[:, :], in_=rec[:])
                        bcc = p2.tile([64, 512], fp32, tag="bc64", name="bcc")
                        nc.sync.dma_start(out=bcc[:], in_=scr[0:1, :].partition_broadcast(64))
                        nc.vector.tensor_mul(
                            out=ctxc[hh * 64:(hh + 1) * 64, a, qs],
                            in0=cp[0:64, :], in1=bcc[:])

            # ---------- Phase 6: cross out-proj + residual + LN2 ----------
            wco = load_w(wd["co"])
            proj_add_residual(wco, ctxc, COB)
            c_bf = p1.tile([P, HT, QL], bf16, tag="act_bf", name="c_bf")
            layer_norm(CLG, CLB, out_bf=c_bf)

            # ---------- Phase 7: FFN (chunk-outer: each weight block loads once) --
            for ch in range(FT // FC):
                inters = [p2.tile([P, HT, 512], bf16, tag="xs", bufs=2,
                                  name=f"inter{i}") for i in range(QN)]
                for mi in range(FC):
                    m = ch * FC + mi
                    wi = p3.tile([P, HT, P], bf16, tag="wi", bufs=4, name="wi")
                    nc.sync.dma_start(out=wi[:], in_=w_i[:, m, :, :])
                    for qn in range(QN):
                        qs = slice(qn * 512, (qn + 1) * 512)
                        ps = psA.tile([P, 512], fp32, tag="mm", name="psi")
                        for kt in range(HT):
                            nc.tensor.matmul(ps[:], wi[:, kt, :], c_bf[:, kt, qs],
                                             start=(kt == 0), stop=(kt == HT - 1))
                        nc.scalar.activation(out=inters[qn][:, mi, :], in_=ps[:],
                                             func=mybir.ActivationFunctionType.Gelu,
                                             bias=col(IB + m), scale=1.0)
                for mo in range(HT):
                    wo = p2.tile([P, FC, P], bf16, tag="wo", bufs=4, name="wo")
                    nc.sync.dma_start(out=wo[:],
                                      in_=w_o[:, mo, ch * FC:(ch + 1) * FC, :])
                    for qn in range(QN):
                        qs = slice(qn * 512, (qn + 1) * 512)
                        ps = psA.tile([P, 512], fp32, tag="mm", name="pso2")
                        for kt in range(FC):
                            nc.tensor.matmul(ps[:], wo[:, kt, :],
                                             inters[qn][:, kt, :],
                                             start=(kt == 0), stop=(kt == FC - 1))
                        if ch == 0:
                            nc.vector.scalar_tensor_tensor(
                                out=work[:, mo, qs], in0=ps[:], scalar=col(OB + mo),
                                in1=work[:, mo, qs],
                                op0=mybir.AluOpType.add, op1=mybir.AluOpType.add)
                        else:
                            nc.vector.tensor_add(out=work[:, mo, qs], in0=ps[:],
                                                 in1=work[:, mo, qs])

            layer_norm(OLG, OLB)
            for qn in range(QN):
                qs = slice(qn * 512, (qn + 1) * 512)
                for j in range(HT):
                    nc.sync.dma_start(out=yT[j * P:(j + 1) * P, qs],
                                      in_=work[:, j, qs])

    nc.compile()
    return nc, band_kq


def _get_program(er):
    key = int(er)
    if key not in _CACHE:
        _CACHE[key] = _build(key)
    return _CACHE[key]


def _tr_w(w, ktt, mbt):
    """[K, M] -> [P, mb, kt, 128] so per-mb DMAs read 2KB/partition chunks."""
    return np.ascontiguousarray(
        np.asarray(w, np.float32).reshape(ktt, P, mbt, P).transpose(1, 2, 0, 3)
    ).astype(BF)


def build_in_maps(inp, band_kq, er):
    x = inp["x"].astype(np.float32)
    B, S_, H_ = x.shape

    # host-side shared staging
    wcast = {}
    for n in ("sq", "sk", "sv", "so", "cq", "ck", "cv", "co"):
        wcast[n] = _tr_w(inp[n + "_w"], HT, HT)
    wcast["i"] = _tr_w(inp["i_w"], HT, FT)
    wcast["o"] = _tr_w(inp["o_w"], FT, HT)

    so_b_eff = inp["so_b"].astype(np.float32) + \
        inp["sv_b"].astype(np.float32) @ inp["so_w"].astype(np.float32)
    co_b_eff = inp["co_b"].astype(np.float32) + \
        inp["cv_b"].astype(np.float32) @ inp["co_w"].astype(np.float32)
    pvec = np.zeros((P, PCOLS), np.float32)

    def pack(colbase, vec):
        v = np.asarray(vec, np.float32).reshape(-1, P)  # [k, 128]
        pvec[:, colbase:colbase + v.shape[0]] = v.T

    pack(QB8, inp["sq_b"].astype(np.float32) * 0.125)
    pack(KB, inp["sk_b"])
    pack(SOB, so_b_eff)
    pack(SLG, inp["sln_g"]); pack(SLB, inp["sln_b"])
    pack(CQB8, inp["cq_b"].astype(np.float32) * 0.125)
    pack(CKB, inp["ck_b"])
    pack(COB, co_b_eff)
    pack(CLG, inp["cln_g"]); pack(CLB, inp["cln_b"])
    pack(IB, inp["i_b"])
    pack(OB, inp["o_b"])
    pack(OLG, inp["oln_g"]); pack(OLB, inp["oln_b"])
    pvec[:, ONECOL] = 1.0

    tags = inp["emb_table"].astype(np.float32)[
        np.asarray(inp["ent_ids"]).astype(np.int64)]  # [T, H]
    assert tags.shape[0] == T, f"program compiled for {T} tags, got {tags.shape[0]}"
    tagsT = np.zeros((H, 64), np.float32)
    tagsT[:, :tags.shape[0]] = tags.T
    tagsT3 = np.ascontiguousarray(
        tagsT.reshape(HT, P, 64).transpose(1, 0, 2)).astype(BF)
    ones = np.ones((P, 1), BF)

    # band tiles (e^mask) in local (rotated) coords, per half
    nb = max(len(band_kq), 1)
    band_h = np.ones((2, P, nb, 512), np.float32)
    if band_kq and er > 0:
        for i, (kt, qn) in enumerate(band_kq):
            k_rot = kt * P + np.arange(P)[:, None]
            q_rot = qn * 512 + np.arange(512)[None, :]
            d = k_rot - q_rot
            m0 = (np.abs(d) <= er).astype(np.float32)
            m1 = np.where(k_rot >= S_ - QL,
                          (np.abs(d - S_) <= er).astype(np.float32), m0)
            band_h[0, :, i, :] = np.exp(m0)
            band_h[1, :, i, :] = np.exp(m1)
    band_h = band_h.astype(BF)

    in_maps = []
    for c in range(8):
        b, half = divmod(c, 2)
        xt = x[b].T  # [H, S]
        rot = np.concatenate([xt[:, half * QL:], xt[:, :half * QL]], axis=1)
        xt3 = np.ascontiguousarray(
            rot.reshape(HT, P, S_).transpose(1, 0, 2)).astype(BF)
        xh3 = np.ascontiguousarray(
            rot[:, :QL].reshape(HT, P, QL).transpose(1, 0, 2))
        in_maps.append({
            "xT": xt3,
            "xh": xh3,
            "w_sq": wcast["sq"], "w_sk": wcast["sk"], "w_sv": wcast["sv"],
            "w_so": wcast["so"], "w_cq": wcast["cq"], "w_ck": wcast["ck"],
            "w_cv": wcast["cv"], "w_co": wcast["co"],
            "w_i": wcast["i"], "w_o": wcast["o"],
            "pvec": pvec, "tagsT": tagsT3, "ones": ones,
            "band": np.ascontiguousarray(band_h[half]),
        })
    return in_maps


def kernel(**inputs):
    inp = {k: np.asarray(v) for k, v in inputs.items()}
    x = inp["x"]
    B, S_, H_ = x.shape
    er = int(inp["ent_range"])
    nc, band_kq = _get_program(er)
    in_maps = build_in_maps(inp, band_kq, er)

    res = run_bass_kernel_spmd(nc, in_maps, core_ids=list(range(8)))
    out = np.empty((B, S_, H_), np.float32)
    for c in range(8):
        b, half = divmod(c, 2)
        out[b, half * QL:(half + 1) * QL, :] = res.results[c]["yT"].T
    return out
